# revision 1
# baseline (speedup 1.0000x reference)
"""Self-contained Trainium2 Bass kernel for nn_MultiHeadAttention_80599356276988.

Strategy: tensor-parallel over heads (2 heads/core x 8 cores), flash-style
causal attention with scores kept on-chip, AllToAll to redistribute
head-major activations to row-major blocks, per-core output projection of
512 rows. All matmuls run as float32r (full-rate fp32 streaming mode).
"""
import sys

sys.path.insert(0, "/opt/trn_rl_repo")
import numpy as np
from contextlib import ExitStack

import concourse.bass as bass
import concourse.mybir as mybir
import concourse.tile as tile
from concourse import bacc
from concourse.bass_utils import run_bass_kernel_spmd

F32 = mybir.dt.float32
F32R = mybir.dt.float32r
BF16 = mybir.dt.bfloat16
EXP = mybir.ActivationFunctionType.Exp

B, T, C = 2, 2048, 1024
H, D = 16, 64
NCORES = 8
HPC = H // NCORES        # heads per core = 2
DPC = HPC * D            # dims per core = 128
N = B * T                # 4096 flat rows
RPC = N // NCORES        # output rows per core = 512
SCALE = float(C) ** -0.5
TC4 = T // 512           # 4 t-chunks of 512 per batch
NT8 = N // 512           # 8 flat t-chunks of 512

_CACHE = {}


def build_nc():
    nc = bacc.Bacc(num_devices=NCORES)

    XT = nc.dram_tensor("xt", [C, N], F32R, kind="ExternalInput")
    WQ3 = nc.dram_tensor("wq3", [128, C], F32R, kind="ExternalInput")
    WK3 = nc.dram_tensor("wk3", [128, C], F32R, kind="ExternalInput")
    WV3 = nc.dram_tensor("wv3", [128, C], F32R, kind="ExternalInput")
    WPT = nc.dram_tensor("wpt8", [C, C], F32R, kind="ExternalInput")
    BIAS = nc.dram_tensor("bias", [1, C], F32, kind="ExternalInput")
    CMASK = nc.dram_tensor("cmask", [4, 128, 512], F32R, kind="ExternalInput")
    ONES = nc.dram_tensor("ones16", [128, 16], F32R, kind="ExternalInput")
    IDENT = nc.dram_tensor("ident", [128, 64], F32R, kind="ExternalInput")
    OUT = nc.dram_tensor("out", [RPC, C], F32, kind="ExternalOutput")

    a2a_in = nc.dram_tensor("a2a_in", [NCORES, DPC, RPC], F32R)
    a2a_out = nc.dram_tensor("a2a_out", [NCORES, DPC, RPC], F32R)

    with tile.TileContext(nc) as tc, ExitStack() as ctx:
        consts = ctx.enter_context(tc.tile_pool(name="consts", bufs=1))
        qkv = ctx.enter_context(tc.tile_pool(name="qkv", bufs=1))

        # --- constants (tiles allocated now; DMAs emitted at first use so the
        # phase-A X loads win the DMA queues early) ---
        wq_sb = consts.tile([128, C], F32R, tag="wq")
        wk_sb = consts.tile([128, C], F32R, tag="wk")
        wv_sb = consts.tile([128, C], F32R, tag="wv")
        # per-chunk slices: the cc=0 matmul only needs the first 128 columns,
        # so the PE can start before the rest of the weights land
        for cc in range(8):
            sl = slice(cc * 128, (cc + 1) * 128)
            nc.sync.dma_start(wq_sb[:, sl], WQ3[:, sl])
            nc.sync.dma_start(wk_sb[:, sl], WK3[:, sl])
            nc.sync.dma_start(wv_sb[:, sl], WV3[:, sl])
        wpt_sb = [consts.tile([128, C], F32R, tag=f"wpt{j}", name=f"wpt{j}")
                  for j in range(8)]
        cmask_sb = consts.tile([128, 4 * 512], F32R, tag="cmask")
        ident_sb = consts.tile([128, 64], F32R, tag="ident")
        bias_sb = consts.tile([128, C], F32, tag="bias")

        # --- persistent activations ---
        # KTz: per-head zero-padded K^T (other head's rows zeroed) so scores
        # matmuls run at K=128 with the same PE tile shape as everything else
        # (PE pays ~300ns per tile-shape switch between consecutive matmuls).
        QT = qkv.tile([128, N], BF16, tag="QT")
        KTz = qkv.tile([128, 2 * N], BF16, tag="KTz")
        VS = qkv.tile([128, 4 * 1040], F32R, tag="VS")  # per pair: 16 stiles x (64 d + 1 ones)

        # ---------------- Phase A: QKV projections ----------------
        with tc.tile_pool(name="phA", bufs=1) as phA, \
             tc.tile_pool(name="xtp", bufs=2) as xtp, \
             tc.tile_pool(name="psA", bufs=2, space="PSUM") as psA:
            VT = phA.tile([128, N], F32R, tag="VT")
            nc.vector.memset(KTz[64:128, 0:N], 0.0)
            nc.vector.memset(KTz[0:64, N:2 * N], 0.0)
            for t8 in range(NT8):
                xts = []
                for cc in range(8):
                    xt_t = xtp.tile([128, 512], F32R, tag=f"xt{cc}")
                    nc.sync.dma_start(
                        xt_t[:], XT[cc * 128:(cc + 1) * 128, t8 * 512:(t8 + 1) * 512])
                    xts.append(xt_t)
                for w_sb, dst, nm in ((wq_sb, QT, "q"), (wk_sb, None, "k"), (wv_sb, VT, "v")):
                    pp = psA.tile([128, 512], F32, tag=f"pp{nm}")
                    for cc in range(8):
                        nc.tensor.matmul(
                            pp[:], w_sb[:, cc * 128:(cc + 1) * 128], xts[cc][:],
                            start=(cc == 0), stop=(cc == 7))
                    if nm == "k":
                        cols = slice(t8 * 512, (t8 + 1) * 512)
                        nc.vector.tensor_copy(KTz[0:64, cols], pp[0:64, :])
                        nc.vector.tensor_copy(
                            KTz[64:128, N + t8 * 512:N + (t8 + 1) * 512],
                            pp[64:128, :])
                    else:
                        nc.vector.tensor_copy(dst[:, t8 * 512:(t8 + 1) * 512], pp[:])

            # ---------------- Phase A2: V transpose + ones columns ----------------
            nc.sync.dma_start(ident_sb[:], IDENT[:])
            for b in range(B):
                for h in range(HPC):
                    pair = b * HPC + h
                    base = pair * 1040
                    nc.sync.dma_start(VS[:, base + 64:base + 1040:65], ONES[:, 0:16])
                    for st in range(16):
                        ptr = psA.tile([128, 64], F32R, tag="ptr")
                        nc.tensor.transpose(
                            ptr[:],
                            VT[h * 64:(h + 1) * 64,
                               b * T + st * 128:b * T + (st + 1) * 128],
                            ident_sb[h * 64:(h + 1) * 64, 0:64])
                        nc.vector.tensor_copy(
                            VS[:, base + st * 65:base + st * 65 + 64], ptr[:])

        # ---------------- Phase B: attention ----------------
        # Stile-outer loop: consecutive matmuls share the same stationary
        # operand (KT stile for scores, VS stile for AV), avoiding the PE
        # shape/weight-switch penalty on every instruction. Per-t4 psum
        # accumulators (4 banks) hold the AV chains; scores go through
        # 2-bank psum group tiles with one exp per group.
        with tc.tile_pool(name="phB", bufs=2) as phB, \
             tc.tile_pool(name="psB", bufs=2, space="PSUM") as psB, \
             tc.tile_pool(name="drB", bufs=2, space="DRAM") as drB:
            for k in range(4):
                nc.sync.dma_start(cmask_sb[:, k * 512:(k + 1) * 512], CMASK[k])
            for j in range(8):
                nc.sync.dma_start(wpt_sb[j][:], WPT[j * 128:(j + 1) * 128, :])
            nc.sync.dma_start(bias_sb[:], BIAS[0:1, :].to_broadcast((128, C)))
            LOOKAHEAD = 3

            def emit_scores(b, h, t4, g):
                bcol = b * T
                qcol = bcol + t4 * 512
                psg = psB.tile([128, 1024], F32, tag="psg", name="psg", bufs=3)
                for half in range(2):
                    st = 2 * g + half
                    nc.tensor.matmul(
                        psg[:, half * 512:(half + 1) * 512],
                        KTz[:, h * N + bcol + st * 128:
                            h * N + bcol + (st + 1) * 128],
                        QT[:, qcol:qcol + 512],
                        start=True, stop=True)
                Pg = phB.tile([128, 1024], F32R, tag="P", name="Pg",
                              bufs=LOOKAHEAD + 2)
                nc.scalar.activation(Pg[:], psg[:], EXP, scale=SCALE)
                for half in range(2):
                    st = 2 * g + half
                    if st >= 4 * t4:
                        k = st - 4 * t4
                        sl = Pg[:, half * 512:(half + 1) * 512]
                        nc.vector.tensor_mul(
                            sl, sl, cmask_sb[:, k * 512:(k + 1) * 512])
                return Pg

            state = {}

            def emit_av(item):
                b, h, t4, g, Pg = item
                pair = b * HPC + h
                vbase = pair * 1040
                last = 4 * t4 + 3
                if g == 0:
                    state[(b, h, t4)] = psB.tile([65, 512], F32, tag="pav",
                                                 name="pav", bufs=2)
                pav = state[(b, h, t4)]
                for half in range(2):
                    st = 2 * g + half
                    nc.tensor.matmul(
                        pav[0:65, :],
                        VS[:, vbase + st * 65:vbase + st * 65 + 65],
                        Pg[:, half * 512:(half + 1) * 512],
                        start=(st == 0), stop=(st == last))
                if 2 * g + 1 == last:
                    # evacuate psum fast so the pav bank frees quickly
                    pav_sb = phB.tile([65, 512], F32, tag="pavsb",
                                      name="pav_sb")
                    nc.vector.tensor_copy(pav_sb[:], pav[0:65, :])
                    rt = phB.tile([128, 512], F32, tag="rt", name="rt")
                    nc.vector.reciprocal(rt[64:65, :], pav_sb[64:65, :])
                    rd = drB.tile([1, 512], F32, tag="rd", name="rd")
                    nc.sync.dma_start(rd[:], rt[64:65, :])
                    Rb = phB.tile([64, 512], F32, tag="Rb", name="Rb")
                    nc.sync.dma_start(Rb[:], rd[:].to_broadcast((64, 512)))
                    act_t = phB.tile([64, 512], F32R, tag="act", name="act_t")
                    nc.vector.tensor_mul(act_t[:], pav_sb[0:64, :], Rb[:])
                    nc.sync.dma_start(
                        a2a_in[b * TC4 + t4, h * 64:h * 64 + 64, :], act_t[:])

            for b in range(B):
                for h in range(HPC):
                    for t4 in range(TC4):
                        pending = []
                        for g in range((4 * t4 + 4) // 2):
                            Pg = emit_scores(b, h, t4, g)
                            pending.append((b, h, t4, g, Pg))
                            if len(pending) > LOOKAHEAD:
                                emit_av(pending.pop(0))
                        for item in pending:
                            emit_av(item)

            # ---------------- Phase C: AllToAll ----------------
            nc.gpsimd.collective_compute(
                "AllToAll", mybir.AluOpType.bypass,
                replica_groups=[list(range(NCORES))],
                ins=[a2a_in[:]], outs=[a2a_out[:]])

        # ---------------- Phase D: output projection ----------------
        with tc.tile_pool(name="phD", bufs=2) as phD, \
             tc.tile_pool(name="rvp", bufs=1) as rvp, \
             tc.tile_pool(name="psD", bufs=2, space="PSUM") as psD:
            rv = []
            for j in range(8):
                r = rvp.tile([128, 512], F32R, tag=f"rv{j}", name=f"rv{j}")
                nc.sync.dma_start(r[:], a2a_out[j])
                rv.append(r)
            for mt in range(4):
                for oc in range(2):
                    po = psD.tile([128, 512], F32, tag="po")
                    for j in range(8):
                        nc.tensor.matmul(
                            po[:],
                            rv[j][:, mt * 128:(mt + 1) * 128],
                            wpt_sb[j][:, oc * 512:(oc + 1) * 512],
                            start=(j == 0), stop=(j == 7))
                    ot = phD.tile([128, 512], F32, tag="ot")
                    nc.vector.tensor_add(ot[:], po[:], bias_sb[:, oc * 512:(oc + 1) * 512])
                    nc.sync.dma_start(
                        OUT[mt * 128:(mt + 1) * 128, oc * 512:(oc + 1) * 512], ot[:])

    nc.compile()
    return nc


def prep_in_maps(X, Wq, Wk, Wv, Wp, bp):
    X = np.asarray(X, dtype=np.float32)
    Wq = np.asarray(Wq, dtype=np.float32)
    Wk = np.asarray(Wk, dtype=np.float32)
    Wv = np.asarray(Wv, dtype=np.float32)
    Wp = np.asarray(Wp, dtype=np.float32)
    bp = np.asarray(bp, dtype=np.float32)

    XT = np.ascontiguousarray(X.reshape(N, C).T)            # [C, N]
    WPT = np.ascontiguousarray(Wp.T)                        # [C(concat), C(out)]
    bias = np.ascontiguousarray(bp.reshape(1, C))

    cmask = np.zeros((4, 128, 512), dtype=np.float32)
    for k in range(4):
        p = np.arange(128)[:, None]
        f = np.arange(512)[None, :]
        cmask[k] = (128 * k + p <= f).astype(np.float32)
    ones16 = np.ones((128, 16), dtype=np.float32)
    ident = np.ascontiguousarray(np.tile(np.eye(64, dtype=np.float32), (2, 1)))

    def w3(Wfull, i):
        Wc = Wfull[HPC * i:HPC * i + HPC].reshape(DPC, C)   # [m, c]
        WT = np.ascontiguousarray(Wc.T)                     # [c, m]
        return np.ascontiguousarray(
            WT.reshape(8, 128, DPC).transpose(1, 0, 2).reshape(128, C))

    in_maps = []
    for i in range(NCORES):
        in_maps.append({
            "xt": XT,
            "wq3": w3(Wq, i),
            "wk3": w3(Wk, i),
            "wv3": w3(Wv, i),
            "wpt8": WPT,
            "bias": bias,
            "cmask": cmask,
            "ones16": ones16,
            "ident": ident,
        })
    return in_maps


def run(inputs, trace=False, trace_kwargs=None):
    if "nc" not in _CACHE:
        _CACHE["nc"] = build_nc()
    nc = _CACHE["nc"]
    in_maps = prep_in_maps(**inputs)
    res = run_bass_kernel_spmd(
        nc, in_maps, list(range(NCORES)), trace=trace,
        **(trace_kwargs or {}))
    out = np.concatenate([res.results[i]["out"] for i in range(NCORES)], axis=0)
    return out.reshape(B, T, C), res


def kernel(**inputs) -> np.ndarray:
    out, _ = run(inputs, trace=False)
    return out



# revision 4
# speedup vs baseline: 1.2820x; 1.2820x over previous
"""Self-contained Trainium2 Bass kernel for nn_MultiHeadAttention_80599356276988.

Strategy: tensor-parallel over heads (2 heads/core x 8 cores), flash-style
causal attention with scores kept on-chip, head-split AllToAll (two
collectives so the first overlaps attention compute), per-core output
projection of 512 rows. All operands are bf16 (host-side cast) which halves
HBM traffic and runs every matmul at 1 cycle/row.
"""
import sys

sys.path.insert(0, "/opt/trn_rl_repo")
import numpy as np
from contextlib import ExitStack

import concourse.bass as bass
import concourse.mybir as mybir
import concourse.tile as tile
from concourse import bacc
from concourse.bass_utils import run_bass_kernel_spmd

F32 = mybir.dt.float32
BF16 = mybir.dt.bfloat16
EXP = mybir.ActivationFunctionType.Exp

B, T, C = 2, 2048, 1024
H, D = 16, 64
NCORES = 8
HPC = H // NCORES        # heads per core = 2
DPC = HPC * D            # dims per core = 128
N = B * T                # 4096 flat rows
RPC = N // NCORES        # output rows per core = 512
SCALE = float(C) ** -0.5
TC4 = T // 512           # 4 t-chunks of 512 per batch
NT8 = N // 512           # 8 flat t-chunks of 512

_CACHE = {}


def build_nc():
    nc = bacc.Bacc(num_devices=NCORES)

    # X pre-tiled on host: XT8[t8][cc] = [128, 512] bf16, contiguous per tile
    XT8 = nc.dram_tensor("xt8", [NT8, 8, 128, 512], BF16, kind="ExternalInput")
    WQ3 = nc.dram_tensor("wq3", [128, C], BF16, kind="ExternalInput")
    WK3 = nc.dram_tensor("wk3", [128, C], BF16, kind="ExternalInput")
    WV3 = nc.dram_tensor("wv3", [128, C], BF16, kind="ExternalInput")
    WPT = nc.dram_tensor("wpt8", [C, C], BF16, kind="ExternalInput")
    BIAS = nc.dram_tensor("bias", [1, C], F32, kind="ExternalInput")
    CMASK = nc.dram_tensor("cmask", [4, 128, 512], BF16, kind="ExternalInput")
    ONES = nc.dram_tensor("ones16", [128, 16], BF16, kind="ExternalInput")
    IDENT = nc.dram_tensor("ident", [128, 128], BF16, kind="ExternalInput")
    OUT = nc.dram_tensor("out", [RPC, C], F32, kind="ExternalOutput")

    # per-head collective payloads: [dest chunk, 64 dims, 512 rows]
    a2a_in = [nc.dram_tensor(f"a2a_in{h}", [NCORES, 64, RPC], BF16)
              for h in range(HPC)]
    a2a_out = [nc.dram_tensor(f"a2a_out{h}", [NCORES, 64, RPC], BF16)
               for h in range(HPC)]

    with tile.TileContext(nc) as tc, ExitStack() as ctx:
        consts = ctx.enter_context(tc.tile_pool(name="consts", bufs=1))
        qkv = ctx.enter_context(tc.tile_pool(name="qkv", bufs=1))

        # constant tiles (DMAs ordered so phase-A X loads win the queues)
        wq_sb = consts.tile([128, C], BF16, tag="wq")
        wk_sb = consts.tile([128, C], BF16, tag="wk")
        wv_sb = consts.tile([128, C], BF16, tag="wv")
        ident_sb = consts.tile([128, 128], BF16, tag="ident")
        wpt_sb = [consts.tile([128, C], BF16, tag=f"wpt{j}", name=f"wpt{j}")
                  for j in range(8)]
        cmask_sb = consts.tile([128, 4 * 512], BF16, tag="cmask")
        bias_sb = consts.tile([128, C], F32, tag="bias")

        # persistent activations, all [head-dim, token] layout
        QT = qkv.tile([128, N], BF16, tag="QT")
        # KTz: per-head zero-padded K^T (other head's rows zeroed) so scores
        # matmuls run at K=128 with a uniform PE tile shape.
        KTz = qkv.tile([128, 2 * N], BF16, tag="KTz")
        VS = qkv.tile([128, 4 * 1040], BF16, tag="VS")  # 16 stiles x (64 d + 1 ones)

        # ---------------- Phase A: QKV projections + V transpose ----------------
        with tc.tile_pool(name="phA", bufs=1) as phA, \
             tc.tile_pool(name="xtp", bufs=3) as xtp, \
             tc.tile_pool(name="psA", bufs=2, space="PSUM") as psA, \
             tc.tile_pool(name="psT", bufs=2, space="PSUM") as psT:
            VT = phA.tile([128, N], BF16, tag="VT")
            nc.vector.memset(KTz[64:128, 0:N], 0.0)
            nc.vector.memset(KTz[0:64, N:2 * N], 0.0)
            nc.sync.dma_start(ident_sb[:], IDENT[:])

            def load_xt(t8):
                xts = []
                for cc in range(8):
                    xt_t = xtp.tile([128, 512], BF16, tag=f"xt{cc}")
                    nc.sync.dma_start(xt_t[:], XT8[t8, cc])
                    xts.append(xt_t)
                return xts

            xts_cur = load_xt(0)
            # full-tile weight loads (contiguous, after first X chunk)
            nc.sync.dma_start(wq_sb[:], WQ3[:])
            nc.sync.dma_start(wk_sb[:], WK3[:])
            nc.sync.dma_start(wv_sb[:], WV3[:])
            for b in range(B):
                for h in range(HPC):
                    base = (b * HPC + h) * 1040
                    nc.sync.dma_start(VS[:, base + 64:base + 1040:65],
                                      ONES[:, 0:16])

            for t8 in range(NT8):
                xts = xts_cur
                xts_cur = load_xt(t8 + 1) if t8 + 1 < NT8 else None
                for w_sb, nm in ((wq_sb, "q"), (wk_sb, "k"), (wv_sb, "v")):
                    pp = psA.tile([128, 512], F32, tag=f"pp{nm}")
                    for cc in range(8):
                        nc.tensor.matmul(
                            pp[:], w_sb[:, cc * 128:(cc + 1) * 128], xts[cc][:],
                            start=(cc == 0), stop=(cc == 7))
                    cols = slice(t8 * 512, (t8 + 1) * 512)
                    if nm == "q":
                        nc.vector.tensor_copy(QT[:, cols], pp[:])
                    elif nm == "k":
                        nc.vector.tensor_copy(KTz[0:64, cols], pp[0:64, :])
                        nc.vector.tensor_copy(
                            KTz[64:128, N + t8 * 512:N + (t8 + 1) * 512],
                            pp[64:128, :])
                    else:
                        nc.vector.tensor_copy(VT[:, cols], pp[:])
                # V transpose for this chunk: 4 x [128,128] blocks (both heads)
                b = t8 // 4
                for q in range(4):
                    st = (t8 % 4) * 4 + q          # 128-tok stile within batch
                    ptr = psT.tile([128, 128], BF16, tag="ptr")
                    nc.tensor.transpose(
                        ptr[:],
                        VT[:, b * T + st * 128:b * T + (st + 1) * 128],
                        ident_sb[:])
                    for h in range(HPC):
                        base = (b * HPC + h) * 1040
                        nc.vector.tensor_copy(
                            VS[:, base + st * 65:base + st * 65 + 64],
                            ptr[:, h * 64:(h + 1) * 64])

        # ---------------- Phase B: attention ----------------
        with tc.tile_pool(name="phB", bufs=2) as phB, \
             tc.tile_pool(name="psB", bufs=2, space="PSUM") as psB:
            for k in range(4):
                nc.sync.dma_start(cmask_sb[:, k * 512:(k + 1) * 512], CMASK[k])
            nc.sync.dma_start(bias_sb[:], BIAS[0:1, :].to_broadcast((128, C)))
            for j in range(8):
                nc.sync.dma_start(wpt_sb[j][:], WPT[j * 128:(j + 1) * 128, :])
            LOOKAHEAD = 3

            def emit_scores(b, h, t4, g):
                bcol = b * T
                qcol = bcol + t4 * 512
                psg = psB.tile([128, 1024], F32, tag="psg", name="psg", bufs=3)
                for half in range(2):
                    st = 2 * g + half
                    nc.tensor.matmul(
                        psg[:, half * 512:(half + 1) * 512],
                        KTz[:, h * N + bcol + st * 128:
                            h * N + bcol + (st + 1) * 128],
                        QT[:, qcol:qcol + 512],
                        start=True, stop=True)
                Pg = phB.tile([128, 1024], BF16, tag="P", name="Pg",
                              bufs=LOOKAHEAD + 2)
                nc.scalar.activation(Pg[:], psg[:], EXP, scale=SCALE)
                for half in range(2):
                    st = 2 * g + half
                    if st >= 4 * t4:
                        k = st - 4 * t4
                        sl = Pg[:, half * 512:(half + 1) * 512]
                        nc.vector.tensor_mul(
                            sl, sl, cmask_sb[:, k * 512:(k + 1) * 512])
                return Pg

            state = {}

            def emit_av(item):
                b, h, t4, g, Pg = item
                pair = b * HPC + h
                vbase = pair * 1040
                last = 4 * t4 + 3
                if g == 0:
                    state[(b, h, t4)] = psB.tile([65, 512], F32, tag="pav",
                                                 name="pav", bufs=2)
                pav = state[(b, h, t4)]
                for half in range(2):
                    st = 2 * g + half
                    nc.tensor.matmul(
                        pav[0:65, :],
                        VS[:, vbase + st * 65:vbase + st * 65 + 65],
                        Pg[:, half * 512:(half + 1) * 512],
                        start=(st == 0), stop=(st == last))
                if 2 * g + 1 == last:
                    # evacuate psum fast so the pav bank frees quickly
                    act_sb = phB.tile([64, 512], BF16, tag="actsb",
                                      name="act_sb")
                    nc.vector.tensor_copy(act_sb[:], pav[0:64, :])
                    sums = phB.tile([1, 512], F32, tag="sums", name="sums")
                    nc.vector.tensor_copy(sums[:], pav[64:65, :])
                    rec = phB.tile([1, 512], F32, tag="rec", name="rec")
                    nc.vector.reciprocal_approx_fast(out=rec[:], in_=sums[:])
                    rb = phB.tile([64, 512], BF16, tag="rb", name="rb")
                    nc.vector.tensor_copy(rb[0:1, :], rec[:])
                    nc.gpsimd.partition_broadcast(rb[:], rb[0:1, :],
                                                  channels=64)
                    act_t = phB.tile([64, 512], BF16, tag="act", name="act_t")
                    nc.vector.tensor_mul(act_t[:], act_sb[:], rb[:])
                    nc.sync.dma_start(a2a_in[h][b * TC4 + t4], act_t[:])

            for h in range(HPC):
                for b in range(B):
                    for t4 in range(TC4):
                        pending = []
                        for g in range((4 * t4 + 4) // 2):
                            Pg = emit_scores(b, h, t4, g)
                            pending.append((b, h, t4, g, Pg))
                            if len(pending) > LOOKAHEAD:
                                emit_av(pending.pop(0))
                        for item in pending:
                            emit_av(item)
                # per-head AllToAll: the h=0 collective overlaps h=1 compute
                nc.gpsimd.collective_compute(
                    "AllToAll", mybir.AluOpType.bypass,
                    replica_groups=[list(range(NCORES))],
                    ins=[a2a_in[h][:]], outs=[a2a_out[h][:]])

        # ---------------- Phase D: output projection ----------------
        with tc.tile_pool(name="phD", bufs=2) as phD, \
             tc.tile_pool(name="rvp", bufs=1) as rvp, \
             tc.tile_pool(name="psD", bufs=2, space="PSUM") as psD:
            rv = []
            for j in range(8):
                r = rvp.tile([128, 512], BF16, tag=f"rv{j}", name=f"rv{j}")
                nc.sync.dma_start(r[0:64, :], a2a_out[0][j])
                nc.sync.dma_start(r[64:128, :], a2a_out[1][j])
                rv.append(r)
            for mt in range(4):
                for oc in range(2):
                    po = psD.tile([128, 512], F32, tag="po")
                    for j in range(8):
                        nc.tensor.matmul(
                            po[:],
                            rv[j][:, mt * 128:(mt + 1) * 128],
                            wpt_sb[j][:, oc * 512:(oc + 1) * 512],
                            start=(j == 0), stop=(j == 7))
                    ot = phD.tile([128, 512], F32, tag="ot")
                    nc.vector.tensor_add(ot[:], po[:], bias_sb[:, oc * 512:(oc + 1) * 512])
                    nc.sync.dma_start(
                        OUT[mt * 128:(mt + 1) * 128, oc * 512:(oc + 1) * 512], ot[:])

    nc.compile()
    return nc


def prep_in_maps(X, Wq, Wk, Wv, Wp, bp):
    bf16 = mybir.dt.np(BF16)
    X = np.asarray(X, dtype=np.float32)
    Wq = np.asarray(Wq, dtype=np.float32)
    Wk = np.asarray(Wk, dtype=np.float32)
    Wv = np.asarray(Wv, dtype=np.float32)
    Wp = np.asarray(Wp, dtype=np.float32)
    bp = np.asarray(bp, dtype=np.float32)

    XT = X.reshape(N, C).T                                   # [C, N]
    # pre-tiled [t8][cc][128, 512], contiguous per tile, bf16
    XT8 = np.ascontiguousarray(
        XT.reshape(8, 128, NT8, 512).transpose(2, 0, 1, 3)).astype(bf16)
    WPT = np.ascontiguousarray(Wp.T).astype(bf16)            # [C(concat), C(out)]
    bias = np.ascontiguousarray(bp.reshape(1, C))

    cmask = np.zeros((4, 128, 512), dtype=np.float32)
    for k in range(4):
        p = np.arange(128)[:, None]
        f = np.arange(512)[None, :]
        cmask[k] = (128 * k + p <= f).astype(np.float32)
    cmask = cmask.astype(bf16)
    ones16 = np.ones((128, 16), dtype=bf16)
    ident = np.eye(128, dtype=np.float32).astype(bf16)

    def w3(Wfull, i):
        Wc = Wfull[HPC * i:HPC * i + HPC].reshape(DPC, C)   # [m, c]
        WT = np.ascontiguousarray(Wc.T)                     # [c, m]
        return np.ascontiguousarray(
            WT.reshape(8, 128, DPC).transpose(1, 0, 2).reshape(128, C)
        ).astype(bf16)

    in_maps = []
    for i in range(NCORES):
        in_maps.append({
            "xt8": XT8,
            "wq3": w3(Wq, i),
            "wk3": w3(Wk, i),
            "wv3": w3(Wv, i),
            "wpt8": WPT,
            "bias": bias,
            "cmask": cmask,
            "ones16": ones16,
            "ident": ident,
        })
    return in_maps


def run(inputs, trace=False, trace_kwargs=None):
    if "nc" not in _CACHE:
        _CACHE["nc"] = build_nc()
    nc = _CACHE["nc"]
    in_maps = prep_in_maps(**inputs)
    res = run_bass_kernel_spmd(
        nc, in_maps, list(range(NCORES)), trace=trace,
        **(trace_kwargs or {}))
    out = np.concatenate([res.results[i]["out"] for i in range(NCORES)], axis=0)
    return out.reshape(B, T, C), res


def kernel(**inputs) -> np.ndarray:
    out, _ = run(inputs, trace=False)
    return out
